# revision 5
# baseline (speedup 1.0000x reference)
"""Causal self-attention TRN2 Bass kernel v2 (B=4, T=2048, C=1024, H=16, D=64).

Sharding: 8 cores = 4 batches x 2 head-groups (8 heads each). All matmuls in
bf16 (same PE rate as fp32r, half the LDWEIGHTS/SBUF/DMA), fp32 PSUM/exp.

Fully SBUF-resident (no DRAM scratch). Pipelined by head-pair: the ACT-bound
attention of pair pr overlaps with "filler" PE work (V projection, QK of pair
pr+1, output projection) interleaved 2 matmuls per kb step, so the PE array
never idles long enough to re-throttle (HAM) and the exp stream on ScalarE
runs back-to-back.

Per head-pair pr (heads 2pr, 2pr+1 of this core's 8):
  S^T[k,q] = K^T.T Q^T per 128-key block (2 heads packed via tile_position),
  exp on ACT (scale=1/8) -> bf16 P; causal mask zeroed on GpSimd affine_select;
  AV with ones-augmented V (psum rows 0:64 = O^T, 64:128 = sums);
  divide via DVE reciprocal_approx_fast + mult -> oT (bf16).
Projection y^T = W_proj^T oT (partial over this core's heads) -> DRAM f32.
Host: y[b] = (yT[2b] + yT[2b+1]).T
"""

import numpy as np
from collections import deque
from contextlib import ExitStack

import ml_dtypes
import concourse.bass as bass
import concourse.tile as tile
from concourse import bacc, mybir
from concourse.bass import ts
from concourse.bass_utils import run_bass_kernel_spmd

N_CORES = 8
B, T, C, H, D = 4, 2048, 1024, 16, 64
CB = C // 128           # 8 contraction blocks
NKB = T // 128          # 16 key blocks
NQC = T // 512          # 4 query chunks
NEG = -1.0e9

F32 = mybir.dt.float32
BF16 = mybir.dt.bfloat16
AF = mybir.ActivationFunctionType
OP = mybir.AluOpType

_CACHE = {}


class Fillers:
    """Queue of small matmul-emission segments pumped between attention steps."""

    def __init__(self):
        self.q = deque()

    def add_chain(self, fns, per=2):
        for i in range(0, len(fns), per):
            self.q.append(fns[i : i + per])

    def pump(self, n):
        for _ in range(n):
            if not self.q:
                return
            for fn in self.q.popleft():
                fn()

    def drain(self):
        while self.q:
            self.pump(1)


def _build(reps=1, debug=False):
    nc = bacc.Bacc("TRN2", target_bir_lowering=False, debug=False, num_devices=N_CORES)

    xT = nc.dram_tensor("xT", [C, T], BF16, kind="ExternalInput").ap()
    w_qk = nc.dram_tensor("w_qk", [C, 1024], BF16, kind="ExternalInput").ap()
    w_v = nc.dram_tensor("w_v", [C, 512], BF16, kind="ExternalInput").ap()
    w_pr = nc.dram_tensor("w_pr", [512, C], BF16, kind="ExternalInput").ap()
    b_qk = nc.dram_tensor("b_qk", [1024], F32, kind="ExternalInput").ap()
    b_v = nc.dram_tensor("b_v", [128, 512], F32, kind="ExternalInput").ap()
    b_pr = nc.dram_tensor("b_pr", [C], F32, kind="ExternalInput").ap()
    yT = nc.dram_tensor("yT", [C, T], F32, kind="ExternalOutput").ap()
    if debug:
        dbg_v = nc.dram_tensor("dbg_v", [128, NKB * 8 * 2 * 64], BF16, kind="ExternalOutput").ap()
        dbg_qk = nc.dram_tensor("dbg_qk", [128, 2 * T], BF16, kind="ExternalOutput").ap()
        dbg_oT = nc.dram_tensor("dbg_oT", [128, 4 * T], BF16, kind="ExternalOutput").ap()
        dbg_p = nc.dram_tensor("dbg_p", [128, 2 * 512], BF16, kind="ExternalOutput").ap()

    xT_r = xT.rearrange("(cb p) t -> p cb t", p=128)
    w_qk_r = w_qk.rearrange("(cb p) m -> p cb m", p=128)
    w_v_r = w_v.rearrange("(cb p) m -> p cb m", p=128)
    w_pr_r = w_pr.rearrange("(pb p) m -> p pb m", p=128)
    b_qk_r = b_qk.rearrange("(m p) -> p m", p=128)
    b_pr_r = b_pr.rearrange("(m p) -> p m", p=128)
    yT_r = yT.rearrange("(m p) t -> p m t", p=128)

    with tile.TileContext(nc) as tc:
        with ExitStack() as ctx:
            pers = ctx.enter_context(tc.tile_pool(name="pers", bufs=1))
            qk_p = ctx.enter_context(tc.tile_pool(name="qk", bufs=8))
            p_p = ctx.enter_context(tc.tile_pool(name="pp", bufs=4))
            rec_p = ctx.enter_context(tc.tile_pool(name="rec", bufs=4))
            st_p = ctx.enter_context(tc.tile_pool(name="st", bufs=4))
            ps_f = ctx.enter_context(tc.tile_pool(name="ps_f", bufs=2, space="PSUM"))
            ps_s = ctx.enter_context(tc.tile_pool(name="ps_s", bufs=2, space="PSUM"))
            ps_o = ctx.enter_context(tc.tile_pool(name="ps_o", bufs=2, space="PSUM"))

            # constants / weights (one-time)
            b_qk_sb = pers.tile([128, 8], F32)
            nc.sync.dma_start(b_qk_sb[:], b_qk_r)
            b_v_sb = pers.tile([128, 512], F32)
            nc.sync.dma_start(b_v_sb[:], b_v)
            b_pr_sb = pers.tile([128, 8], F32)
            nc.sync.dma_start(b_pr_sb[:], b_pr_r)
            w_qk_sb = pers.tile([128, CB, 1024], BF16)
            nc.sync.dma_start(w_qk_sb[:], w_qk_r)
            w_v_sb = pers.tile([128, CB, 512], BF16)
            nc.sync.dma_start(w_v_sb[:], w_v_r)
            w_pr_sb = pers.tile([128, 4, 1024], BF16)
            nc.sync.dma_start(w_pr_sb[:], w_pr_r)

            x_sb = pers.tile([128, CB, T], BF16)
            # v_sb: per key-block, per head: 64 ones cols then 64 V cols,
            # so the AV psum has sums at rows 0:64 (base partition 0 --
            # reciprocal_approx_fast breaks on base-64 inputs) and O^T at 64:128.
            v_sb = pers.tile([128, NKB, 8, 2, 64], BF16)
            nc.gpsimd.memset(v_sb[:, :, :, 0], 1.0)
            oT = pers.tile([128, 4, T], BF16)
            dbg_p_sb = pers.tile([128, 2, 512], BF16, name="dbg_p_sb") if debug else None

            def v_chain(rep, tb):
                ps = ps_f.tile([128, 512], F32, tag="f", name=f"v_{rep}_{tb}")
                fns = [
                    (lambda cb=cb, ps=ps, tb=tb: nc.tensor.matmul(
                        ps[:], x_sb[:, cb, tb * 128 : (tb + 1) * 128],
                        w_v_sb[:, cb], start=(cb == 0), stop=(cb == CB - 1)))
                    for cb in range(CB)
                ]

                def drain(ps=ps, tb=tb):
                    for pr in range(4):
                        nc.vector.tensor_tensor(
                            v_sb[:, tb, 2 * pr : 2 * pr + 2, 1],
                            ps[:, ts(pr, 128)].rearrange("p (g c) -> p g c", g=2),
                            b_v_sb[:, ts(pr, 128)].rearrange("p (g c) -> p g c", g=2),
                            OP.add,
                        )

                fns.append(drain)
                return fns

            def qk_chain(rep, dst, m, tch):
                ps = ps_f.tile([128, 512], F32, tag="f", name=f"qk_{rep}_{m}_{tch}")
                fns = [
                    (lambda cb=cb, ps=ps, m=m, tch=tch: nc.tensor.matmul(
                        ps[:], w_qk_sb[:, cb, ts(m, 128)], x_sb[:, cb, ts(tch, 512)],
                        start=(cb == 0), stop=(cb == CB - 1)))
                    for cb in range(CB)
                ]

                def drain(ps=ps, dst=dst, m=m, tch=tch):
                    nc.vector.tensor_scalar(
                        dst[:, ts(tch, 512)], ps[:], b_qk_sb[:, m : m + 1], None,
                        op0=OP.add,
                    )

                fns.append(drain)
                return fns

            def proj_chain(rep, m, tch):
                ps = ps_f.tile([128, 512], F32, tag="f", name=f"y_{rep}_{m}_{tch}")
                fns = [
                    (lambda pb=pb, ps=ps, m=m, tch=tch: nc.tensor.matmul(
                        ps[:], w_pr_sb[:, pb, ts(m, 128)], oT[:, pb, ts(tch, 512)],
                        start=(pb == 0), stop=(pb == 3)))
                    for pb in range(4)
                ]

                def drain(ps=ps, m=m, tch=tch):
                    st = st_p.tile([128, 512], F32, tag="st", name=f"ys_{rep}_{m}_{tch}")
                    nc.vector.tensor_scalar(
                        st[:], ps[:], b_pr_sb[:, m : m + 1], None, op0=OP.add,
                    )
                    nc.sync.dma_start(yT_r[:, m, ts(tch, 512)], st[:])

                fns.append(drain)
                return fns

            def qkt_tile(rep, kind, pr):
                return qk_p.tile([128, T], BF16, tag="qkt", name=f"{kind}_{rep}_{pr}")

            carry = None  # (qT0, kT0) of next rep; its head was emitted in our tail
            for _rep in range(reps):
                fil = Fillers()

                if carry is None:
                    if _rep == 0:
                        for tch in range(4):
                            nc.sync.dma_start(x_sb[:, :, ts(tch, 512)], xT_r[:, :, ts(tch, 512)])
                    qt0 = qkt_tile(_rep, "qT", 0)
                    kt0 = qkt_tile(_rep, "kT", 0)
                    # fill block: V for kb 0..7, then QK for pair 0
                    for tb in range(8):
                        for fn in v_chain(_rep, tb):
                            fn()
                    for tch in range(4):
                        for fn in qk_chain(_rep, qt0, 0, tch):
                            fn()
                        for fn in qk_chain(_rep, kt0, 4, tch):
                            fn()
                else:
                    qt0, kt0 = carry
                    carry = None

                # q/k tiles (pool cycles 8 bufs; pr's tiles are dead once
                # pr+1's attention starts, so the next rep may reuse them)
                qts = [qt0] + [qkt_tile(_rep, "qT", pr) for pr in range(1, 4)]
                kts = [kt0] + [qkt_tile(_rep, "kT", pr) for pr in range(1, 4)]

                for pr in range(4):
                    qT, kT = qts[pr], kts[pr]
                    if pr == 3 and _rep + 1 < reps:
                        # prefetch next rep's x while this rep finishes
                        # (x_sb is dead after pr2's QK fillers)
                        for tch in range(4):
                            nc.sync.dma_start(
                                x_sb[:, :, ts(tch, 512)], xT_r[:, :, ts(tch, 512)])
                    # enqueue fillers consumed during this pair's attention
                    if pr == 0:
                        for tb in range(8, 16):
                            fil.add_chain(v_chain(_rep, tb))
                    if pr < 3:
                        for tch in range(4):
                            fil.add_chain(qk_chain(_rep, qts[pr + 1], pr + 1, tch))
                            fil.add_chain(qk_chain(_rep, kts[pr + 1], 4 + pr + 1, tch))

                    steps_left = sum(4 * qc + 4 for qc in range(4))
                    for qc in range(NQC):
                        nkb = 4 * qc + 4
                        pso = [
                            ps_o.tile([128, 512], F32, tag="o", name=f"o_{_rep}_{pr}_{qc}_{j}")
                            for j in (0, 1)
                        ]
                        ss = [None] * nkb
                        pt = [None] * nkb

                        def s_step(kb, qc=qc, pr=pr, qT=qT, kT=kT, ss=ss):
                            r = kb - 4 * qc
                            qlo = 128 * r if r > 0 else 0
                            s = ps_s.tile([128, 2, 512], F32, tag="s",
                                          name=f"s_{_rep}_{pr}_{qc}_{kb}")
                            for j in (0, 1):
                                pb = j * 64
                                nc.tensor.matmul(
                                    s[:, j, qlo:512],
                                    kT[pb : pb + 64, ts(kb, 128)],
                                    qT[pb : pb + 64, qc * 512 + qlo : (qc + 1) * 512],
                                    start=True, stop=True, tile_position=(pb, 0),
                                )
                            ss[kb] = s

                        def x_step(kb, qc=qc, pr=pr, ss=ss, pt=pt):
                            r = kb - 4 * qc
                            qlo = 128 * r if r > 0 else 0
                            p_t = p_p.tile([128, 2, 512], BF16, tag="p",
                                           name=f"p_{_rep}_{pr}_{qc}_{kb}")
                            nc.scalar.activation(
                                p_t[:, :, qlo:512], ss[kb][:, :, qlo:512],
                                AF.Exp, scale=0.125,
                            )
                            if r >= 0:
                                # zero P where q < k inside the diagonal block
                                nc.gpsimd.affine_select(
                                    out=p_t[:, :, qlo : qlo + 128],
                                    in_=p_t[:, :, qlo : qlo + 128],
                                    compare_op=OP.is_ge, fill=0.0,
                                    base=0, pattern=[[0, 2], [1, 128]],
                                    channel_multiplier=-1,
                                )
                            pt[kb] = p_t

                        def av_step(kb, qc=qc, pr=pr, ss=ss, pt=pt, pso=pso, nkb=nkb):
                            r = kb - 4 * qc
                            qlo = 128 * r if r > 0 else 0
                            p_t = pt[kb]
                            if debug and pr == 0 and qc == 0 and kb == 0:
                                nc.vector.tensor_copy(dbg_p_sb[:], p_t[:])
                            for j in (0, 1):
                                nc.tensor.matmul(
                                    pso[j][:, qlo:512],
                                    v_sb[:, kb, 2 * pr + j],
                                    p_t[:, j, qlo:512],
                                    start=(kb == 0), stop=(kb == nkb - 1),
                                )

                        s_step(0)
                        x_step(0)
                        fil.pump(1)
                        s_step(1)
                        x_step(1)
                        # at pr2, hold back ~10 segments (tch3's QK chains,
                        # not read until qc3) so pr3's early chunks -- which
                        # have no proj fillers yet -- keep the PE fed
                        reserve = 10 if pr == 2 else 0
                        for kb in range(nkb):
                            if kb + 2 < nkb:
                                s_step(kb + 2)
                                x_step(kb + 2)
                            want = 2 if len(fil.q) > steps_left + reserve else 1
                            fil.pump(want)
                            av_step(kb)
                            steps_left -= 1

                        for j in (0, 1):
                            rec = rec_p.tile([64, 512], F32, tag="rec",
                                             name=f"rec_{_rep}_{pr}_{qc}_{j}")
                            nc.vector.reciprocal_approx_fast(
                                out=rec[:], in_=pso[j][0:64, :])
                            nc.vector.tensor_tensor(
                                oT[j * 64 : (j + 1) * 64, pr, ts(qc, 512)],
                                pso[j][64:128, :], rec[:], OP.mult,
                            )
                        if pr == 3 and qc < 3:
                            for m in range(8):
                                fil.add_chain(proj_chain(_rep, m, qc))
                    if pr == 3:
                        # tail: proj of qc3 interleaved with the NEXT rep's
                        # head (V chains + QK pair0), so the PE never waits
                        # on ps_f/DVE recycling at the rep seam
                        tail = [proj_chain(_rep, m, 3) for m in range(8)]
                        if _rep + 1 < reps:
                            qt0n = qkt_tile(_rep + 1, "qT", 0)
                            kt0n = qkt_tile(_rep + 1, "kT", 0)
                            nxt = [v_chain(_rep + 1, tb) for tb in range(8)]
                            for tch in range(4):
                                nxt.append(qk_chain(_rep + 1, qt0n, 0, tch))
                                nxt.append(qk_chain(_rep + 1, kt0n, 4, tch))
                            carry = (qt0n, kt0n)
                            inter = []
                            while tail or nxt:
                                if tail:
                                    inter.append(tail.pop(0))
                                for _ in range(2):
                                    if nxt:
                                        inter.append(nxt.pop(0))
                            tail = inter
                        for ch in tail:
                            fil.add_chain(ch)
                        fil.drain()

                if debug:
                    nc.sync.dma_start(dbg_v, v_sb[:].rearrange("p a b c d -> p (a b c d)"))
                    nc.sync.dma_start(dbg_qk.rearrange("p (a t) -> p a t", a=2)[:, 0], qts[0][:])
                    nc.sync.dma_start(dbg_qk.rearrange("p (a t) -> p a t", a=2)[:, 1], kts[0][:])
                    nc.sync.dma_start(dbg_oT, oT[:].rearrange("p a t -> p (a t)"))
                    nc.sync.dma_start(dbg_p, dbg_p_sb[:].rearrange("p a t -> p (a t)"))

    nc.compile()
    return nc


def _bf16(a):
    return np.ascontiguousarray(a.astype(ml_dtypes.bfloat16))


def _in_maps(x, W_attn, b_attn, W_proj, b_proj):
    maps = []
    for b in range(B):
        for g in range(2):
            cs = slice(g * 512, (g + 1) * 512)
            maps.append({
                "xT": _bf16(x[b].T),
                "w_qk": _bf16(np.concatenate(
                    [W_attn[:, cs], W_attn[:, 1024 + cs.start : 1024 + cs.stop]], axis=1)),
                "w_v": _bf16(W_attn[:, 2048 + cs.start : 2048 + cs.stop]),
                "w_pr": _bf16(W_proj[cs, :]),
                "b_qk": np.ascontiguousarray(np.concatenate(
                    [b_attn[cs], b_attn[1024 + cs.start : 1024 + cs.stop]])),
                "b_v": np.ascontiguousarray(
                    np.tile(b_attn[2048 + cs.start : 2048 + cs.stop][None, :], (128, 1))),
                "b_pr": np.ascontiguousarray(b_proj),
            })
    return maps


def kernel(x, W_attn, b_attn, W_proj, b_proj):
    x = np.asarray(x, dtype=np.float32)
    W_attn = np.asarray(W_attn, dtype=np.float32)
    b_attn = np.asarray(b_attn, dtype=np.float32)
    W_proj = np.asarray(W_proj, dtype=np.float32)
    b_proj = np.asarray(b_proj, dtype=np.float32)

    if "nc" not in _CACHE:
        _CACHE["nc"] = _build()
    nc = _CACHE["nc"]

    maps = _in_maps(x, W_attn, b_attn, W_proj, b_proj)
    last_exc = None
    for attempt in range(3):
        try:
            res = run_bass_kernel_spmd(nc, maps, core_ids=list(range(N_CORES)))
            break
        except Exception as exc:  # transient device wedges recover on retry
            last_exc = exc
            if attempt == 2:
                raise
            import time as _time
            _time.sleep(5)
    y = np.empty((B, T, C), dtype=np.float32)
    for b in range(B):
        y[b] = (res.results[2 * b]["yT"] + res.results[2 * b + 1]["yT"]).T
    return y



# revision 12
# speedup vs baseline: 1.2290x; 1.2290x over previous
"""Causal self-attention TRN2 Bass kernel v2 (B=4, T=2048, C=1024, H=16, D=64).

Sharding: 8 cores = 4 batches x 2 head-groups (8 heads each). All matmuls in
bf16 (same PE rate as fp32r, half the LDWEIGHTS/SBUF/DMA), fp32 PSUM/exp.

Fully SBUF-resident (no DRAM scratch). Pipelined by head-pair: the ACT-bound
attention of pair pr overlaps with "filler" PE work (V projection, QK of pair
pr+1, output projection) interleaved 2 matmuls per kb step, so the PE array
never idles long enough to re-throttle (HAM) and the exp stream on ScalarE
runs back-to-back.

Per head-pair pr (heads 2pr, 2pr+1 of this core's 8):
  S^T[k,q] = K^T.T Q^T per 128-key block (2 heads packed via tile_position),
  exp on ACT (scale=1/8) -> bf16 P; causal mask zeroed on GpSimd affine_select;
  AV with ones-augmented V (psum rows 0:64 = O^T, 64:128 = sums);
  divide via DVE reciprocal_approx_fast + mult -> oT (bf16).
Projection y^T = W_proj^T oT (partial over this core's heads) -> DRAM f32.
Host: y[b] = (yT[2b] + yT[2b+1]).T
"""

import numpy as np
from collections import deque
from contextlib import ExitStack

import ml_dtypes
import concourse.bass as bass
import concourse.tile as tile
from concourse import bacc, mybir
from concourse.bass import ts
from concourse.bass_utils import run_bass_kernel_spmd

N_CORES = 8
B, T, C, H, D = 4, 2048, 1024, 16, 64
CB = C // 128           # 8 contraction blocks
NKB = T // 128          # 16 key blocks
NQC = T // 512          # 4 query chunks
NEG = -1.0e9

F32 = mybir.dt.float32
BF16 = mybir.dt.bfloat16
AF = mybir.ActivationFunctionType
OP = mybir.AluOpType

_CACHE = {}


class Fillers:
    """Queue of small matmul-emission segments pumped between attention steps."""

    def __init__(self):
        self.q = deque()

    def add_chain(self, fns, per=2):
        for i in range(0, len(fns), per):
            self.q.append(fns[i : i + per])

    def pump(self, n):
        for _ in range(n):
            if not self.q:
                return
            for fn in self.q.popleft():
                fn()

    def drain(self):
        while self.q:
            self.pump(1)


def _build(reps=1, debug=False, nobias=True):
    nc = bacc.Bacc("TRN2", target_bir_lowering=False, debug=False, num_devices=N_CORES)

    xT = nc.dram_tensor("xT", [C, T], BF16, kind="ExternalInput").ap()
    w_qk = nc.dram_tensor("w_qk", [C, 1024], BF16, kind="ExternalInput").ap()
    w_v = nc.dram_tensor("w_v", [C, 512], BF16, kind="ExternalInput").ap()
    w_pr = nc.dram_tensor("w_pr", [512, C], BF16, kind="ExternalInput").ap()
    b_qk = nc.dram_tensor("b_qk", [1024], F32, kind="ExternalInput").ap()
    b_v = nc.dram_tensor("b_v", [128, 512], F32, kind="ExternalInput").ap()
    b_pr = nc.dram_tensor("b_pr", [C], F32, kind="ExternalInput").ap()
    yT = nc.dram_tensor("yT", [C, T], F32, kind="ExternalOutput").ap()
    if debug:
        dbg_v = nc.dram_tensor("dbg_v", [128, NKB * 8 * 2 * 64], BF16, kind="ExternalOutput").ap()
        dbg_qk = nc.dram_tensor("dbg_qk", [128, 2 * T], BF16, kind="ExternalOutput").ap()
        dbg_oT = nc.dram_tensor("dbg_oT", [128, 4 * T], BF16, kind="ExternalOutput").ap()
        dbg_p = nc.dram_tensor("dbg_p", [128, 2 * 512], BF16, kind="ExternalOutput").ap()

    xT_r = xT.rearrange("(cb p) t -> p cb t", p=128)
    w_qk_r = w_qk.rearrange("(cb p) m -> p cb m", p=128)
    w_v_r = w_v.rearrange("(cb p) m -> p cb m", p=128)
    w_pr_r = w_pr.rearrange("(pb p) m -> p pb m", p=128)
    b_qk_r = b_qk.rearrange("(m p) -> p m", p=128)
    b_pr_r = b_pr.rearrange("(m p) -> p m", p=128)
    yT_r = yT.rearrange("(m p) t -> p m t", p=128)

    with tile.TileContext(nc) as tc:
        with ExitStack() as ctx:
            pers = ctx.enter_context(tc.tile_pool(name="pers", bufs=1))
            qk_p = ctx.enter_context(tc.tile_pool(name="qk", bufs=8))
            p_p = ctx.enter_context(tc.tile_pool(name="pp", bufs=4))
            rec_p = ctx.enter_context(tc.tile_pool(name="rec", bufs=4))
            st_p = ctx.enter_context(tc.tile_pool(name="st", bufs=4))
            ps_f = ctx.enter_context(tc.tile_pool(name="ps_f", bufs=2, space="PSUM"))
            ps_s = ctx.enter_context(tc.tile_pool(name="ps_s", bufs=2, space="PSUM"))
            ps_o = ctx.enter_context(tc.tile_pool(name="ps_o", bufs=2, space="PSUM"))

            # constants / weights (one-time); bias tiles skipped when nobias
            if not nobias:
                b_qk_sb = pers.tile([128, 8], F32)
                nc.sync.dma_start(b_qk_sb[:], b_qk_r)
                b_v_sb = pers.tile([128, 512], F32)
                nc.sync.dma_start(b_v_sb[:], b_v)
                b_pr_sb = pers.tile([128, 8], F32)
                nc.sync.dma_start(b_pr_sb[:], b_pr_r)
            w_qk_sb = pers.tile([128, CB, 1024], BF16)
            nc.sync.dma_start(w_qk_sb[:], w_qk_r)
            w_v_sb = pers.tile([128, CB, 512], BF16)
            nc.sync.dma_start(w_v_sb[:], w_v_r)
            w_pr_sb = pers.tile([128, 4, 1024], BF16)
            nc.sync.dma_start(w_pr_sb[:], w_pr_r)

            x_sb = pers.tile([128, CB, T], BF16)
            # v_sb: per key-block, per head: 64 ones cols then 64 V cols,
            # so the AV psum has sums at rows 0:64 (base partition 0 --
            # reciprocal_approx_fast breaks on base-64 inputs) and O^T at 64:128.
            v_sb = pers.tile([128, NKB, 8, 2, 64], BF16)
            nc.gpsimd.memset(v_sb[:, :, :, 0], 1.0)
            oT = pers.tile([128, 4, T], BF16)
            dbg_p_sb = pers.tile([128, 2, 512], BF16, name="dbg_p_sb") if debug else None

            for _rep in range(reps):
                if _rep == 0:
                    for tch in range(4):
                        nc.sync.dma_start(x_sb[:, :, ts(tch, 512)], xT_r[:, :, ts(tch, 512)])

                fil = Fillers()

                def v_chain(tb):
                    ps = ps_f.tile([128, 512], F32, tag="f", name=f"v_{_rep}_{tb}")
                    fns = [
                        (lambda cb=cb, ps=ps, tb=tb: nc.tensor.matmul(
                            ps[:], x_sb[:, cb, tb * 128 : (tb + 1) * 128],
                            w_v_sb[:, cb], start=(cb == 0), stop=(cb == CB - 1)))
                        for cb in range(CB)
                    ]

                    def drain(ps=ps, tb=tb):
                        if nobias:
                            # zero bias: one strided DVE copy instead of 4
                            # bias-adds (GpSimd can't read PSUM on TRN2 hw)
                            nc.vector.tensor_copy(
                                v_sb[:, tb, :, 1],
                                ps[:].rearrange("p (h c) -> p h c", h=8),
                            )
                            return
                        for pr in range(4):
                            nc.vector.tensor_tensor(
                                v_sb[:, tb, 2 * pr : 2 * pr + 2, 1],
                                ps[:, ts(pr, 128)].rearrange("p (g c) -> p g c", g=2),
                                b_v_sb[:, ts(pr, 128)].rearrange("p (g c) -> p g c", g=2),
                                OP.add,
                            )

                    fns.append(drain)
                    return fns

                def qk_chain(dst, m, tch):
                    ps = ps_f.tile([128, 512], F32, tag="f", name=f"qk_{_rep}_{m}_{tch}")
                    fns = [
                        (lambda cb=cb, ps=ps, m=m, tch=tch: nc.tensor.matmul(
                            ps[:], w_qk_sb[:, cb, ts(m, 128)], x_sb[:, cb, ts(tch, 512)],
                            start=(cb == 0), stop=(cb == CB - 1)))
                        for cb in range(CB)
                    ]

                    def drain(ps=ps, dst=dst, m=m, tch=tch):
                        if nobias:
                            nc.vector.tensor_copy(dst[:, ts(tch, 512)], ps[:])
                            return
                        nc.vector.tensor_scalar(
                            dst[:, ts(tch, 512)], ps[:], b_qk_sb[:, m : m + 1], None,
                            op0=OP.add,
                        )

                    fns.append(drain)
                    return fns

                def proj_chain(m, tch):
                    ps = ps_f.tile([128, 512], F32, tag="f", name=f"y_{_rep}_{m}_{tch}")
                    fns = [
                        (lambda pb=pb, ps=ps, m=m, tch=tch: nc.tensor.matmul(
                            ps[:], w_pr_sb[:, pb, ts(m, 128)], oT[:, pb, ts(tch, 512)],
                            start=(pb == 0), stop=(pb == 3)))
                        for pb in range(4)
                    ]

                    def drain(ps=ps, m=m, tch=tch):
                        st = st_p.tile([128, 512], F32, tag="st", name=f"ys_{_rep}_{m}_{tch}")
                        if nobias:
                            # zero bias: plain PSUM->SBUF copy on the ACT
                            # engine (idle during the projection phase)
                            nc.scalar.copy(st[:], ps[:])
                        else:
                            nc.vector.tensor_scalar(
                                st[:], ps[:], b_pr_sb[:, m : m + 1], None, op0=OP.add,
                            )
                        nc.sync.dma_start(yT_r[:, m, ts(tch, 512)], st[:])

                    fns.append(drain)
                    return fns

                # q/k tiles for all pairs (pool cycles 4 bufs; pr's tiles are
                # dead once pr+1's attention starts, so pr+2 may reuse them)
                qts = [qk_p.tile([128, T], BF16, tag="qkt", name=f"qT_{_rep}_{pr}") for pr in range(4)]
                kts = [qk_p.tile([128, T], BF16, tag="qkt", name=f"kT_{_rep}_{pr}") for pr in range(4)]

                # fill block: V for kb 0..7, then QK for pair 0
                for tb in range(8):
                    for fn in v_chain(tb):
                        fn()
                for tch in range(4):
                    for fn in qk_chain(qts[0], 0, tch):
                        fn()
                    for fn in qk_chain(kts[0], 4, tch):
                        fn()

                for pr in range(4):
                    qT, kT = qts[pr], kts[pr]
                    if pr == 3 and _rep + 1 < reps:
                        # prefetch next rep's x while this rep finishes
                        # (x_sb is dead after pr2's QK fillers)
                        for tch in range(4):
                            nc.sync.dma_start(
                                x_sb[:, :, ts(tch, 512)], xT_r[:, :, ts(tch, 512)])
                    # enqueue fillers consumed during this pair's attention
                    if pr == 0:
                        for tb in range(8, 16):
                            fil.add_chain(v_chain(tb))
                    if pr < 3:
                        for tch in range(4):
                            fil.add_chain(qk_chain(qts[pr + 1], pr + 1, tch))
                            fil.add_chain(qk_chain(kts[pr + 1], 4 + pr + 1, tch))

                    steps_left = sum(4 * qc + 4 for qc in range(4))
                    all_steps = [(qc, kb) for qc in range(NQC)
                                 for kb in range(4 * qc + 4)]
                    sss = {qc: [None] * (4 * qc + 4) for qc in range(NQC)}
                    pts = {qc: [None] * (4 * qc + 4) for qc in range(NQC)}
                    psos = {}

                    def s_step(qc, kb, pr=pr, qT=qT, kT=kT):
                        r = kb - 4 * qc
                        qlo = 128 * r if r > 0 else 0
                        s = ps_s.tile([128, 2, 512], F32, tag="s",
                                      name=f"s_{_rep}_{pr}_{qc}_{kb}")
                        for j in (0, 1):
                            pb = j * 64
                            nc.tensor.matmul(
                                s[:, j, qlo:512],
                                kT[pb : pb + 64, ts(kb, 128)],
                                qT[pb : pb + 64, qc * 512 + qlo : (qc + 1) * 512],
                                start=True, stop=True, tile_position=(pb, 0),
                            )
                        sss[qc][kb] = s

                    def x_step(qc, kb, pr=pr):
                        r = kb - 4 * qc
                        qlo = 128 * r if r > 0 else 0
                        p_t = p_p.tile([128, 2, 512], BF16, tag="p",
                                       name=f"p_{_rep}_{pr}_{qc}_{kb}")
                        nc.scalar.activation(
                            p_t[:, :, qlo:512], sss[qc][kb][:, :, qlo:512],
                            AF.Exp, scale=0.125,
                        )
                        if r >= 0:
                            # zero P where q < k inside the diagonal block
                            nc.gpsimd.affine_select(
                                out=p_t[:, :, qlo : qlo + 128],
                                in_=p_t[:, :, qlo : qlo + 128],
                                compare_op=OP.is_ge, fill=0.0,
                                base=0, pattern=[[0, 2], [1, 128]],
                                channel_multiplier=-1,
                            )
                        pts[qc][kb] = p_t

                    def av_step(qc, kb, pr=pr):
                        nkb = 4 * qc + 4
                        r = kb - 4 * qc
                        qlo = 128 * r if r > 0 else 0
                        if kb == 0:
                            psos[qc] = [
                                ps_o.tile([128, 512], F32, tag="o",
                                          name=f"o_{_rep}_{pr}_{qc}_{j}")
                                for j in (0, 1)
                            ]
                        p_t = pts[qc][kb]
                        if debug and pr == 0 and qc == 0 and kb == 0:
                            nc.vector.tensor_copy(dbg_p_sb[:], p_t[:])
                        for j in (0, 1):
                            nc.tensor.matmul(
                                psos[qc][j][:, qlo:512],
                                v_sb[:, kb, 2 * pr + j],
                                p_t[:, j, qlo:512],
                                start=(kb == 0), stop=(kb == nkb - 1),
                            )

                    # pipeline prologue; the lookahead below then stays 2
                    # steps ahead ACROSS qc boundaries so the ACT exp stream
                    # never drains at a chunk switch
                    s_step(0, 0)
                    x_step(0, 0)
                    s_step(0, 1)
                    x_step(0, 1)
                    for idx, (qc, kb) in enumerate(all_steps):
                        nkb = 4 * qc + 4
                        if idx + 2 < len(all_steps):
                            s_step(*all_steps[idx + 2])
                            x_step(*all_steps[idx + 2])
                        av_step(qc, kb)
                        want = 2 if len(fil.q) > steps_left else 1
                        fil.pump(want)
                        steps_left -= 1
                        if kb == nkb - 1:
                            for j in (0, 1):
                                rec = rec_p.tile([64, 512], F32, tag="rec",
                                                 name=f"rec_{_rep}_{pr}_{qc}_{j}")
                                nc.vector.reciprocal_approx_fast(
                                    out=rec[:], in_=psos[qc][j][0:64, :])
                                nc.vector.tensor_tensor(
                                    oT[j * 64 : (j + 1) * 64, pr, ts(qc, 512)],
                                    psos[qc][j][64:128, :], rec[:], OP.mult,
                                )
                            if pr == 3:
                                for m in range(8):
                                    fil.add_chain(proj_chain(m, qc))
                    fil.drain() if pr == 3 else None

                if debug:
                    nc.sync.dma_start(dbg_v, v_sb[:].rearrange("p a b c d -> p (a b c d)"))
                    nc.sync.dma_start(dbg_qk.rearrange("p (a t) -> p a t", a=2)[:, 0], qts[0][:])
                    nc.sync.dma_start(dbg_qk.rearrange("p (a t) -> p a t", a=2)[:, 1], kts[0][:])
                    nc.sync.dma_start(dbg_oT, oT[:].rearrange("p a t -> p (a t)"))
                    nc.sync.dma_start(dbg_p, dbg_p_sb[:].rearrange("p a t -> p (a t)"))

    nc.compile()
    return nc


def _bf16(a):
    return np.ascontiguousarray(a.astype(ml_dtypes.bfloat16))


def _in_maps(x, W_attn, b_attn, W_proj, b_proj):
    maps = []
    for b in range(B):
        for g in range(2):
            cs = slice(g * 512, (g + 1) * 512)
            maps.append({
                "xT": _bf16(x[b].T),
                "w_qk": _bf16(np.concatenate(
                    [W_attn[:, cs], W_attn[:, 1024 + cs.start : 1024 + cs.stop]], axis=1)),
                "w_v": _bf16(W_attn[:, 2048 + cs.start : 2048 + cs.stop]),
                "w_pr": _bf16(W_proj[cs, :]),
                "b_qk": np.ascontiguousarray(np.concatenate(
                    [b_attn[cs], b_attn[1024 + cs.start : 1024 + cs.stop]])),
                "b_v": np.ascontiguousarray(
                    np.tile(b_attn[2048 + cs.start : 2048 + cs.stop][None, :], (128, 1))),
                "b_pr": np.ascontiguousarray(b_proj),
            })
    return maps


def kernel(x, W_attn, b_attn, W_proj, b_proj):
    x = np.asarray(x, dtype=np.float32)
    W_attn = np.asarray(W_attn, dtype=np.float32)
    b_attn = np.asarray(b_attn, dtype=np.float32)
    W_proj = np.asarray(W_proj, dtype=np.float32)
    b_proj = np.asarray(b_proj, dtype=np.float32)

    nobias = not (np.any(b_attn) or np.any(b_proj))
    key = ("nc", nobias)
    if key not in _CACHE:
        _CACHE[key] = _build(nobias=nobias)
    nc = _CACHE[key]

    maps = _in_maps(x, W_attn, b_attn, W_proj, b_proj)
    last_exc = None
    for attempt in range(3):
        try:
            res = run_bass_kernel_spmd(nc, maps, core_ids=list(range(N_CORES)))
            break
        except Exception as exc:  # transient device wedges recover on retry
            last_exc = exc
            if attempt == 2:
                raise
            import time as _time
            _time.sleep(5)
    y = np.empty((B, T, C), dtype=np.float32)
    for b in range(B):
        y[b] = (res.results[2 * b]["yT"] + res.results[2 * b + 1]["yT"]).T
    return y

